# revision 5
# baseline (speedup 1.0000x reference)
"""4-bit column-block-quantized linear (ColBlockQuantizedLinear) on 8 TRN2 cores.

Math:  out[b,o] = scales[o] * (sum_i inp[b,i]*w[o,i] - zeros[o]*rowsum[b])
where w comes from packed bytes q[o,j] (j = i//2): even i -> low nibble l,
odd i -> high nibble h.

Device-side trick: an e3m4 (float8e3) value with bit pattern 0x60|v equals
8 + v/2 exactly for the FULL nibble range v in 0..15 (fixed exponent 2^3,
4 mantissa bits).  So both nibble streams are produced by cheap DVE bit ops
(no ACT casts, no GPSIMD):
    L = (q & 0x0f0f) | 0x6060          (one dual-ALU tensor_scalar)
    H = ((q >> 4) & 0x0f0f) | 0x6060   (two tensor_scalar ops)
and the PE runs mixed-precision matmuls: bf16 activations (stationary,
hi/lo split for ~fp32 accuracy) x e3m4 nibble streams (moving).

With l = 2L-16, h = 2H-16:
    sum_i inp*w = 2*(A_e . L + A_o . H) - 16*rowsum
    out = (2*scales) * (S - (8 + zeros/2)*rowsum)
The rank-1 (8+zeros/2)*rowsum term is a K=4 bf16 correction matmul
(hi/lo-split factors) accumulated into the same PSUM group.

Host byte layout: per core the packed bytes [2048, 1376] are viewed as
uint16 pairs of ADJACENT columns (2m, 2m+1), so the DVE-produced fp8 bytes
land in natural contiguous column order (no strided matmul APs).

Sharding: column-parallel over out_features (1376 rows/core), inputs
replicated; per-core output [16,1376] gathered on host.
"""

import numpy as np
import ml_dtypes

B = 16
I = 4096
O = 11008
NCORES = 8
OS = O // NCORES          # 1376 out-features per core
HALF = I // 2             # 2048 packed-byte rows (contraction dim per stream)
KT = HALF // 128          # 16 contraction tiles
HOS = OS // 2             # 688 u16 columns per packed tile
BLKS = [(0, 512), (512, 512), (1024, 352)]   # psum-bank o-blocks
SPLIT = 2                 # activation bf16 levels (hi/lo)
M = 64 if SPLIT == 2 else 16   # stationary cols (lo group at partition 32)

BF16 = ml_dtypes.bfloat16

_CACHE = {}


def _split_hi_lo(x64):
    """Split float64 array into (hi, lo) bf16 parts: hi+lo ~= x to ~2^-17."""
    hi = x64.astype(BF16)
    lo = (x64 - hi.astype(np.float64)).astype(BF16)
    return hi, lo


def _build_program():
    import concourse.bacc as bacc
    import concourse.mybir as mybir
    import concourse.tile as tile

    dt = mybir.dt
    op = mybir.AluOpType
    nc = bacc.Bacc("TRN2", target_bir_lowering=False)

    q = nc.dram_tensor("q", [HALF, HOS], dt.uint16, kind="ExternalInput")
    statE = nc.dram_tensor("statE", [128, KT * M], dt.bfloat16, kind="ExternalInput")
    statO = nc.dram_tensor("statO", [128, KT * M], dt.bfloat16, kind="ExternalInput")
    corrL = nc.dram_tensor("corrL", [4, M], dt.bfloat16, kind="ExternalInput")
    corrR = nc.dram_tensor("corrR", [4, OS], dt.bfloat16, kind="ExternalInput")
    sc = nc.dram_tensor("sc", [B, OS], dt.float32, kind="ExternalInput")
    out = nc.dram_tensor("out", [B, OS], dt.float32, kind="ExternalOutput")

    with tile.TileContext(nc) as tc:
        with (
            tc.tile_pool(name="consts", bufs=1) as cpool,
            tc.tile_pool(name="qp", bufs=3) as qpool,
            tc.tile_pool(name="wp", bufs=3) as wpool,
            tc.tile_pool(name="op", bufs=2) as opool,
            tc.tile_pool(name="ps", bufs=1, space="PSUM") as pspool,
        ):
            statE_sb = cpool.tile([128, KT * M], dt.bfloat16, name="statE_sb")
            statO_sb = cpool.tile([128, KT * M], dt.bfloat16, name="statO_sb")
            corrL_sb = cpool.tile([4, M], dt.bfloat16, name="corrL_sb")
            corrR_sb = cpool.tile([4, OS], dt.bfloat16, name="corrR_sb")
            sc_sb = cpool.tile([B, OS], dt.float32, name="sc_sb")
            nc.gpsimd.dma_start(statE_sb, statE[:, :])
            nc.gpsimd.dma_start(statO_sb, statO[:, :])

            psums = [
                pspool.tile([M, n], dt.float32, name=f"ps{i}")
                for i, (s, n) in enumerate(BLKS)
            ]

            for kt in range(KT):
                qt = qpool.tile([128, HOS], dt.uint16, name="qt", tag="qt")
                nc.sync.dma_start(qt, q[kt * 128 : (kt + 1) * 128, :])
                lv = wpool.tile([128, HOS], dt.uint16, name="lv", tag="lv")
                t1 = wpool.tile([128, HOS], dt.uint16, name="t1", tag="t1")
                hv = wpool.tile([128, HOS], dt.uint16, name="hv", tag="hv")
                # e3m4 bit trick: 0x60|v == 8 + v/2 exactly for v in 0..15
                nc.vector.tensor_scalar(
                    lv, qt, 0x0F0F, 0x6060, op.bitwise_and, op.bitwise_or
                )
                nc.vector.tensor_scalar(t1, qt, 4, None, op.logical_shift_right)
                nc.vector.tensor_scalar(
                    hv, t1, 0x0F0F, 0x6060, op.bitwise_and, op.bitwise_or
                )
                lv8 = lv.bitcast(dt.float8e3)
                hv8 = hv.bitcast(dt.float8e3)
                ecols = statE_sb[:, kt * M : (kt + 1) * M]
                ocols = statO_sb[:, kt * M : (kt + 1) * M]
                for i, (s, n) in enumerate(BLKS):
                    nc.tensor.matmul(
                        psums[i], ecols, lv8[:, s : s + n],
                        start=(kt == 0), stop=False,
                    )
                    nc.tensor.matmul(
                        psums[i], ocols, hv8[:, s : s + n],
                        start=False, stop=False,
                    )

            nc.gpsimd.dma_start(corrL_sb, corrL[:, :])
            nc.gpsimd.dma_start(corrR_sb, corrR[:, :])
            nc.gpsimd.dma_start(sc_sb, sc[:, :])

            for i, (s, n) in enumerate(BLKS):
                # rank-1 correction: -(8 + zeros/2) * rowsum
                nc.tensor.matmul(
                    psums[i], corrL_sb, corrR_sb[:, s : s + n],
                    start=False, stop=True,
                )
                # chunked output chain for cross-engine pipelining
                CH = 176
                for j, cs in enumerate(range(0, n, CH)):
                    cn = min(CH, n - cs)
                    t0 = opool.tile([B, cn], dt.float32, name="t0", tag=f"t0{i}_{j}")
                    t = opool.tile([B, cn], dt.float32, name="t", tag=f"t{i}_{j}")
                    o = opool.tile([B, cn], dt.float32, name="o", tag=f"o{i}_{j}")
                    ps_c = psums[i][:, cs : cs + cn]
                    # lo-group psum -> sbuf on ACT (one psum read per TT)
                    nc.scalar.activation(
                        t0, ps_c[32:48, :], mybir.ActivationFunctionType.Copy
                    )
                    nc.vector.tensor_tensor(t, ps_c[0:16, :], t0, op.add)
                    nc.vector.tensor_tensor(
                        o, t, sc_sb[:, s + cs : s + cs + cn], op.mult
                    )
                    nc.gpsimd.dma_start(out[:, s + cs : s + cs + cn], o)

    nc.finalize()
    return nc


def _get_program():
    if "nc" not in _CACHE:
        _CACHE["nc"] = _build_program()
    return _CACHE["nc"]


def _host_prep(inp, quant_weight, scales, zeros):
    """Build per-core input maps (layout/precision prep only, no dequant math)."""
    inp64 = np.asarray(inp, dtype=np.float64)
    a_e = inp64[:, 0::2].T.copy()  # [HALF, B] even-i activations (pair with L)
    a_o = inp64[:, 1::2].T.copy()  # [HALF, B] odd-i activations (pair with H)
    e_hi, e_lo = _split_hi_lo(a_e)
    o_hi, o_lo = _split_hi_lo(a_o)

    statE = np.zeros((128, KT * M), dtype=BF16)
    statO = np.zeros((128, KT * M), dtype=BF16)
    for kt in range(KT):
        rows = slice(kt * 128, (kt + 1) * 128)
        statE[:, kt * M : kt * M + 16] = e_hi[rows]
        statO[:, kt * M : kt * M + 16] = o_hi[rows]
        if SPLIT == 2:
            statE[:, kt * M + 32 : kt * M + 48] = e_lo[rows]
            statO[:, kt * M + 32 : kt * M + 48] = o_lo[rows]

    rowsum = inp64.sum(axis=1)  # [B]
    rs_hi, rs_lo = _split_hi_lo(rowsum)
    corrL = np.zeros((4, M), dtype=BF16)
    corrL[0, :16] = rs_hi
    corrL[1, :16] = rs_lo
    corrL[2, :16] = rs_hi
    corrL[3, :16] = rs_lo

    qw = np.asarray(quant_weight)
    scales = np.asarray(scales, dtype=np.float64).reshape(-1)
    zeros = np.asarray(zeros, dtype=np.float64).reshape(-1)

    in_maps = []
    for cidx in range(NCORES):
        rows = slice(cidx * OS, (cidx + 1) * OS)
        qc = qw[rows].astype(np.uint8).T  # [HALF, OS] natural columns
        qu16 = np.ascontiguousarray(qc).view(np.uint16)  # pairs (2m, 2m+1)
        z2 = 8.0 + zeros[rows] / 2.0
        z2_hi, z2_lo = _split_hi_lo(z2)
        corrR = np.zeros((4, OS), dtype=BF16)
        corrR[0] = -z2_hi
        corrR[1] = -z2_hi
        corrR[2] = -z2_lo
        corrR[3] = -z2_lo
        sc_c = np.broadcast_to(
            2.0 * scales[rows].astype(np.float32), (B, OS)
        ).copy()
        in_maps.append(
            {
                "q": qu16,
                "statE": statE,
                "statO": statO,
                "corrL": corrL,
                "corrR": corrR,
                "sc": sc_c,
            }
        )
    return in_maps


def kernel(inp, quant_weight, scales, zeros):
    from concourse.bass_utils import run_bass_kernel_spmd

    nc = _get_program()
    in_maps = _host_prep(inp, quant_weight, scales, zeros)
    res = run_bass_kernel_spmd(nc, in_maps, core_ids=list(range(NCORES)))
    out = np.concatenate(
        [res.results[c]["out"] for c in range(NCORES)], axis=1
    )
    return np.ascontiguousarray(out.astype(np.float32))


# revision 6
# speedup vs baseline: 1.0530x; 1.0530x over previous
"""4-bit column-block-quantized linear (ColBlockQuantizedLinear) on 8 TRN2 cores.

Math:  out[b,o] = scales[o] * (sum_i inp[b,i]*w[o,i] - zeros[o]*rowsum[b])
where w comes from packed bytes q[o,j] (j = i//2): even i -> low nibble l,
odd i -> high nibble h.

Device-side trick: an e3m4 (float8e3) value with bit pattern 0x60|v equals
8 + v/2 exactly for the FULL nibble range v in 0..15 (fixed exponent 2^3,
4 mantissa bits).  So both nibble streams are produced by cheap DVE bit ops
(no ACT casts, no GPSIMD):
    L = (q & 0x0f0f) | 0x6060          (one dual-ALU tensor_scalar)
    H = ((q >> 4) & 0x0f0f) | 0x6060   (two tensor_scalar ops)
and the PE runs mixed-precision matmuls: bf16 activations (stationary,
hi/lo split for ~fp32 accuracy) x e3m4 nibble streams (moving).

With l = 2L-16, h = 2H-16:
    sum_i inp*w = 2*(A_e . L + A_o . H) - 16*rowsum
    out = (2*scales) * (S - (8 + zeros/2)*rowsum)
The rank-1 (8+zeros/2)*rowsum term is a K=4 bf16 correction matmul
(hi/lo-split factors) accumulated into the same PSUM group.

Host byte layout: per core the packed bytes [2048, 1376] are viewed as
uint16 pairs of ADJACENT columns (2m, 2m+1), so the DVE-produced fp8 bytes
land in natural contiguous column order (no strided matmul APs).

Sharding: column-parallel over out_features (1376 rows/core), inputs
replicated; per-core output [16,1376] gathered on host.
"""

import numpy as np
import ml_dtypes

B = 16
I = 4096
O = 11008
NCORES = 8
OS = O // NCORES          # 1376 out-features per core
HALF = I // 2             # 2048 packed-byte rows (contraction dim per stream)
KT = HALF // 128          # 16 contraction tiles
HOS = OS // 2             # 688 u16 columns per packed tile
BLKS = [(0, 512), (512, 512), (1024, 352)]   # psum-bank o-blocks
SPLIT = 2                 # activation bf16 levels (hi/lo)
M = 64 if SPLIT == 2 else 16   # stationary cols (lo group at partition 32)

BF16 = ml_dtypes.bfloat16

_CACHE = {}


def _split_hi_lo(x64):
    """Split float64 array into (hi, lo) bf16 parts: hi+lo ~= x to ~2^-17."""
    hi = x64.astype(BF16)
    lo = (x64 - hi.astype(np.float64)).astype(BF16)
    return hi, lo


def _build_program():
    import concourse.bacc as bacc
    import concourse.mybir as mybir
    import concourse.tile as tile

    dt = mybir.dt
    op = mybir.AluOpType
    nc = bacc.Bacc("TRN2", target_bir_lowering=False)

    q = nc.dram_tensor("q", [HALF, HOS], dt.uint16, kind="ExternalInput")
    statE = nc.dram_tensor("statE", [128, KT * M], dt.bfloat16, kind="ExternalInput")
    statO = nc.dram_tensor("statO", [128, KT * M], dt.bfloat16, kind="ExternalInput")
    corrL = nc.dram_tensor("corrL", [4, M], dt.bfloat16, kind="ExternalInput")
    corrR = nc.dram_tensor("corrR", [4, OS], dt.bfloat16, kind="ExternalInput")
    sc = nc.dram_tensor("sc", [B, OS], dt.float32, kind="ExternalInput")
    out = nc.dram_tensor("out", [B, OS], dt.float32, kind="ExternalOutput")

    with tile.TileContext(nc) as tc:
        with (
            tc.tile_pool(name="consts", bufs=1) as cpool,
            tc.tile_pool(name="qp", bufs=3) as qpool,
            tc.tile_pool(name="wp", bufs=3) as wpool,
            tc.tile_pool(name="op", bufs=2) as opool,
            tc.tile_pool(name="ps", bufs=1, space="PSUM") as pspool,
        ):
            statE_sb = cpool.tile([128, KT * M], dt.bfloat16, name="statE_sb")
            statO_sb = cpool.tile([128, KT * M], dt.bfloat16, name="statO_sb")
            corrL_sb = cpool.tile([4, M], dt.bfloat16, name="corrL_sb")
            corrR_sb = cpool.tile([4, OS], dt.bfloat16, name="corrR_sb")
            sc_sb = cpool.tile([B, OS], dt.float32, name="sc_sb")
            nc.gpsimd.dma_start(statE_sb, statE[:, :])
            nc.gpsimd.dma_start(statO_sb, statO[:, :])

            psums = [
                pspool.tile([M, n], dt.float32, name=f"ps{i}")
                for i, (s, n) in enumerate(BLKS)
            ]

            for kt in range(KT):
                qt = qpool.tile([128, HOS], dt.uint16, name="qt", tag="qt")
                qeng = nc.sync if kt % 2 == 0 else nc.scalar
                qeng.dma_start(qt, q[kt * 128 : (kt + 1) * 128, :])
                lv = wpool.tile([128, HOS], dt.uint16, name="lv", tag="lv")
                t1 = wpool.tile([128, HOS], dt.uint16, name="t1", tag="t1")
                hv = wpool.tile([128, HOS], dt.uint16, name="hv", tag="hv")
                # e3m4 bit trick: 0x60|v == 8 + v/2 exactly for v in 0..15
                nc.vector.tensor_scalar(
                    lv, qt, 0x0F0F, 0x6060, op.bitwise_and, op.bitwise_or
                )
                nc.vector.tensor_scalar(t1, qt, 4, None, op.logical_shift_right)
                nc.vector.tensor_scalar(
                    hv, t1, 0x0F0F, 0x6060, op.bitwise_and, op.bitwise_or
                )
                lv8 = lv.bitcast(dt.float8e3)
                hv8 = hv.bitcast(dt.float8e3)
                ecols = statE_sb[:, kt * M : (kt + 1) * M]
                ocols = statO_sb[:, kt * M : (kt + 1) * M]
                for i, (s, n) in enumerate(BLKS):
                    nc.tensor.matmul(
                        psums[i], ecols, lv8[:, s : s + n],
                        start=(kt == 0), stop=False,
                    )
                    nc.tensor.matmul(
                        psums[i], ocols, hv8[:, s : s + n],
                        start=False, stop=False,
                    )

            nc.gpsimd.dma_start(corrL_sb, corrL[:, :])
            nc.gpsimd.dma_start(corrR_sb, corrR[:, :])
            nc.gpsimd.dma_start(sc_sb, sc[:, :])

            for i, (s, n) in enumerate(BLKS):
                # rank-1 correction: -(8 + zeros/2) * rowsum
                nc.tensor.matmul(
                    psums[i], corrL_sb, corrR_sb[:, s : s + n],
                    start=False, stop=True,
                )
                # chunked output chain for cross-engine pipelining
                CH = 176
                for j, cs in enumerate(range(0, n, CH)):
                    cn = min(CH, n - cs)
                    t0 = opool.tile([B, cn], dt.float32, name="t0", tag=f"t0{i}_{j}")
                    t = opool.tile([B, cn], dt.float32, name="t", tag=f"t{i}_{j}")
                    o = opool.tile([B, cn], dt.float32, name="o", tag=f"o{i}_{j}")
                    ps_c = psums[i][:, cs : cs + cn]
                    # lo-group psum -> sbuf on ACT (one psum read per TT)
                    nc.scalar.activation(
                        t0, ps_c[32:48, :], mybir.ActivationFunctionType.Copy
                    )
                    nc.vector.tensor_tensor(t, ps_c[0:16, :], t0, op.add)
                    nc.vector.tensor_tensor(
                        o, t, sc_sb[:, s + cs : s + cs + cn], op.mult
                    )
                    nc.gpsimd.dma_start(out[:, s + cs : s + cs + cn], o)

    nc.finalize()
    return nc


def _get_program():
    if "nc" not in _CACHE:
        _CACHE["nc"] = _build_program()
    return _CACHE["nc"]


def _host_prep(inp, quant_weight, scales, zeros):
    """Build per-core input maps (layout/precision prep only, no dequant math)."""
    inp64 = np.asarray(inp, dtype=np.float64)
    a_e = inp64[:, 0::2].T.copy()  # [HALF, B] even-i activations (pair with L)
    a_o = inp64[:, 1::2].T.copy()  # [HALF, B] odd-i activations (pair with H)
    e_hi, e_lo = _split_hi_lo(a_e)
    o_hi, o_lo = _split_hi_lo(a_o)

    statE = np.zeros((128, KT * M), dtype=BF16)
    statO = np.zeros((128, KT * M), dtype=BF16)
    for kt in range(KT):
        rows = slice(kt * 128, (kt + 1) * 128)
        statE[:, kt * M : kt * M + 16] = e_hi[rows]
        statO[:, kt * M : kt * M + 16] = o_hi[rows]
        if SPLIT == 2:
            statE[:, kt * M + 32 : kt * M + 48] = e_lo[rows]
            statO[:, kt * M + 32 : kt * M + 48] = o_lo[rows]

    rowsum = inp64.sum(axis=1)  # [B]
    rs_hi, rs_lo = _split_hi_lo(rowsum)
    corrL = np.zeros((4, M), dtype=BF16)
    corrL[0, :16] = rs_hi
    corrL[1, :16] = rs_lo
    corrL[2, :16] = rs_hi
    corrL[3, :16] = rs_lo

    qw = np.asarray(quant_weight)
    scales = np.asarray(scales, dtype=np.float64).reshape(-1)
    zeros = np.asarray(zeros, dtype=np.float64).reshape(-1)

    in_maps = []
    for cidx in range(NCORES):
        rows = slice(cidx * OS, (cidx + 1) * OS)
        qc = qw[rows].astype(np.uint8).T  # [HALF, OS] natural columns
        qu16 = np.ascontiguousarray(qc).view(np.uint16)  # pairs (2m, 2m+1)
        z2 = 8.0 + zeros[rows] / 2.0
        z2_hi, z2_lo = _split_hi_lo(z2)
        corrR = np.zeros((4, OS), dtype=BF16)
        corrR[0] = -z2_hi
        corrR[1] = -z2_hi
        corrR[2] = -z2_lo
        corrR[3] = -z2_lo
        sc_c = np.broadcast_to(
            2.0 * scales[rows].astype(np.float32), (B, OS)
        ).copy()
        in_maps.append(
            {
                "q": qu16,
                "statE": statE,
                "statO": statO,
                "corrL": corrL,
                "corrR": corrR,
                "sc": sc_c,
            }
        )
    return in_maps


def kernel(inp, quant_weight, scales, zeros):
    from concourse.bass_utils import run_bass_kernel_spmd

    nc = _get_program()
    in_maps = _host_prep(inp, quant_weight, scales, zeros)
    res = run_bass_kernel_spmd(nc, in_maps, core_ids=list(range(NCORES)))
    out = np.concatenate(
        [res.results[c]["out"] for c in range(NCORES)], axis=1
    )
    return np.ascontiguousarray(out.astype(np.float32))


# revision 7
# speedup vs baseline: 1.1648x; 1.1061x over previous
"""4-bit column-block-quantized linear (ColBlockQuantizedLinear) on 8 TRN2 cores.

Math:  out[b,o] = scales[o] * (sum_i inp[b,i]*w[o,i] - zeros[o]*rowsum[b])
where w comes from packed bytes q[o,j] (j = i//2): even i -> low nibble l,
odd i -> high nibble h.

Device-side trick: an e3m4 (float8e3) value with bit pattern 0x60|v equals
8 + v/2 exactly for the FULL nibble range v in 0..15 (fixed exponent 2^3,
4 mantissa bits).  So both nibble streams are produced by cheap DVE bit ops
(no ACT casts, no GPSIMD):
    L = (q & 0x0f0f) | 0x6060          (one dual-ALU tensor_scalar)
    H = ((q >> 4) & 0x0f0f) | 0x6060   (two tensor_scalar ops)
and the PE runs mixed-precision matmuls: bf16 activations (stationary) x
e3m4 nibble streams (moving).

With l = 2L-16, h = 2H-16:
    sum_i inp*w = 2*(A_e . L + A_o . H) - 16*rowsum
    out = (2*scales) * (S - (8 + zeros/2)*rowsum)
The rank-1 (8+zeros/2)*rowsum term is a K=4 bf16 correction matmul
(hi/lo-split factors) accumulated into the same PSUM group.

Host byte layout: per core the packed bytes [2048, 1376] are viewed as
uint16 pairs of ADJACENT columns (2m, 2m+1), so the DVE-produced fp8 bytes
land in natural contiguous column order (no strided matmul APs).

DMA: qt weight tiles alternate between the two HWDGE queues (sync/SP and
scalar/ACT); the first tiles are split into half-transfers across both
queues to shorten the ramp (descriptor execution has a ~113ns/descriptor
fixed cost, 16 descriptors per full tile).

Sharding: column-parallel over out_features (1376 rows/core), inputs
replicated; per-core output [16,1376] gathered on host.
"""

import numpy as np
import ml_dtypes

B = 16
I = 4096
O = 11008
NCORES = 8
OS = O // NCORES          # 1376 out-features per core
HALF = I // 2             # 2048 packed-byte rows (contraction dim per stream)
KT = HALF // 128          # 16 contraction tiles
HOS = OS // 2             # 688 u16 columns per packed tile
BLKS = [(0, 512), (512, 512), (1024, 352)]   # psum-bank o-blocks
M = 16                    # stationary cols (single bf16 level)
NSPLIT_RAMP = 4           # first tiles DMA'd as half-transfers on both queues
CH = 176                  # output-stage chunk width

BF16 = ml_dtypes.bfloat16

_CACHE = {}


def _split_hi_lo(x64):
    """Split float64 array into (hi, lo) bf16 parts: hi+lo ~= x to ~2^-17."""
    hi = x64.astype(BF16)
    lo = (x64 - hi.astype(np.float64)).astype(BF16)
    return hi, lo


def _build_program():
    import concourse.bacc as bacc
    import concourse.mybir as mybir
    import concourse.tile as tile

    dt = mybir.dt
    op = mybir.AluOpType
    nc = bacc.Bacc("TRN2", target_bir_lowering=False)

    q = nc.dram_tensor("q", [HALF, HOS], dt.uint16, kind="ExternalInput")
    statE = nc.dram_tensor("statE", [128, KT * M], dt.bfloat16, kind="ExternalInput")
    statO = nc.dram_tensor("statO", [128, KT * M], dt.bfloat16, kind="ExternalInput")
    corrL = nc.dram_tensor("corrL", [4, M], dt.bfloat16, kind="ExternalInput")
    corrR = nc.dram_tensor("corrR", [4, OS], dt.bfloat16, kind="ExternalInput")
    sc = nc.dram_tensor("sc", [B, OS], dt.float32, kind="ExternalInput")
    out = nc.dram_tensor("out", [B, OS], dt.float32, kind="ExternalOutput")

    with tile.TileContext(nc) as tc:
        with (
            tc.tile_pool(name="consts", bufs=1) as cpool,
            tc.tile_pool(name="qp", bufs=4) as qpool,
            tc.tile_pool(name="wp", bufs=3) as wpool,
            tc.tile_pool(name="op", bufs=1) as opool,
            tc.tile_pool(name="ps", bufs=1, space="PSUM") as pspool,
        ):
            statE_sb = cpool.tile([128, KT * M], dt.bfloat16, name="statE_sb")
            statO_sb = cpool.tile([128, KT * M], dt.bfloat16, name="statO_sb")
            corrL_sb = cpool.tile([4, M], dt.bfloat16, name="corrL_sb")
            corrR_sb = cpool.tile([4, OS], dt.bfloat16, name="corrR_sb")
            sc_sb = cpool.tile([B, OS], dt.float32, name="sc_sb")
            nc.gpsimd.dma_start(statE_sb, statE[:, :])
            nc.gpsimd.dma_start(statO_sb, statO[:, :])

            psums = [
                pspool.tile([M, n], dt.float32, name=f"ps{i}")
                for i, (s, n) in enumerate(BLKS)
            ]

            for kt in range(KT):
                qt = qpool.tile([128, HOS], dt.uint16, name="qt", tag="qt")
                r = kt * 128
                if kt < NSPLIT_RAMP:
                    # halve across both HWDGE queues: ~2x faster availability
                    nc.sync.dma_start(qt[0:64, :], q[r : r + 64, :])
                    nc.scalar.dma_start(qt[64:128, :], q[r + 64 : r + 128, :])
                else:
                    qeng = nc.sync if kt % 2 == 0 else nc.scalar
                    qeng.dma_start(qt, q[r : r + 128, :])
                lv = wpool.tile([128, HOS], dt.uint16, name="lv", tag="lv")
                t1 = wpool.tile([128, HOS], dt.uint16, name="t1", tag="t1")
                hv = wpool.tile([128, HOS], dt.uint16, name="hv", tag="hv")
                # e3m4 bit trick: 0x60|v == 8 + v/2 exactly for v in 0..15
                nc.vector.tensor_scalar(
                    lv, qt, 0x0F0F, 0x6060, op.bitwise_and, op.bitwise_or
                )
                nc.vector.tensor_scalar(t1, qt, 4, None, op.logical_shift_right)
                nc.vector.tensor_scalar(
                    hv, t1, 0x0F0F, 0x6060, op.bitwise_and, op.bitwise_or
                )
                lv8 = lv.bitcast(dt.float8e3)
                hv8 = hv.bitcast(dt.float8e3)
                ecols = statE_sb[:, kt * M : (kt + 1) * M]
                ocols = statO_sb[:, kt * M : (kt + 1) * M]
                for i, (s, n) in enumerate(BLKS):
                    nc.tensor.matmul(
                        psums[i], ecols, lv8[:, s : s + n],
                        start=(kt == 0), stop=False,
                    )
                    nc.tensor.matmul(
                        psums[i], ocols, hv8[:, s : s + n],
                        start=False, stop=False,
                    )

            nc.gpsimd.dma_start(corrL_sb, corrL[:, :])
            nc.gpsimd.dma_start(corrR_sb, corrR[:, :])
            nc.gpsimd.dma_start(sc_sb, sc[:, :])

            o = opool.tile([B, OS], dt.float32, name="o")
            for i, (s, n) in enumerate(BLKS):
                # rank-1 correction: -(8 + zeros/2) * rowsum
                nc.tensor.matmul(
                    psums[i], corrL_sb, corrR_sb[:, s : s + n],
                    start=False, stop=True,
                )
                # chunked scale for cross-engine pipelining (one psum read/TT)
                for cs in range(0, n, CH):
                    cn = min(CH, n - cs)
                    nc.vector.tensor_tensor(
                        o[:, s + cs : s + cs + cn],
                        psums[i][:, cs : cs + cn],
                        sc_sb[:, s + cs : s + cs + cn],
                        op.mult,
                    )
            nc.sync.dma_start(out[:, :], o)

    nc.finalize()
    return nc


def _get_program():
    if "nc" not in _CACHE:
        _CACHE["nc"] = _build_program()
    return _CACHE["nc"]


def _host_prep(inp, quant_weight, scales, zeros):
    """Build per-core input maps (layout/precision prep only, no dequant math)."""
    inp64 = np.asarray(inp, dtype=np.float64)
    a_e = inp64[:, 0::2].T.copy()  # [HALF, B] even-i activations (pair with L)
    a_o = inp64[:, 1::2].T.copy()  # [HALF, B] odd-i activations (pair with H)

    statE = np.zeros((128, KT * M), dtype=BF16)
    statO = np.zeros((128, KT * M), dtype=BF16)
    for kt in range(KT):
        rows = slice(kt * 128, (kt + 1) * 128)
        statE[:, kt * M : kt * M + 16] = a_e[rows].astype(BF16)
        statO[:, kt * M : kt * M + 16] = a_o[rows].astype(BF16)

    rowsum = inp64.sum(axis=1)  # [B]
    rs_hi, rs_lo = _split_hi_lo(rowsum)
    corrL = np.zeros((4, M), dtype=BF16)
    corrL[0, :16] = rs_hi
    corrL[1, :16] = rs_lo
    corrL[2, :16] = rs_hi
    corrL[3, :16] = rs_lo

    qw = np.asarray(quant_weight)
    scales = np.asarray(scales, dtype=np.float64).reshape(-1)
    zeros = np.asarray(zeros, dtype=np.float64).reshape(-1)

    in_maps = []
    for cidx in range(NCORES):
        rows = slice(cidx * OS, (cidx + 1) * OS)
        qc = qw[rows].astype(np.uint8).T  # [HALF, OS] natural columns
        qu16 = np.ascontiguousarray(qc).view(np.uint16)  # pairs (2m, 2m+1)
        z2 = 8.0 + zeros[rows] / 2.0
        z2_hi, z2_lo = _split_hi_lo(z2)
        corrR = np.zeros((4, OS), dtype=BF16)
        corrR[0] = -z2_hi
        corrR[1] = -z2_hi
        corrR[2] = -z2_lo
        corrR[3] = -z2_lo
        sc_c = np.broadcast_to(
            2.0 * scales[rows].astype(np.float32), (B, OS)
        ).copy()
        in_maps.append(
            {
                "q": qu16,
                "statE": statE,
                "statO": statO,
                "corrL": corrL,
                "corrR": corrR,
                "sc": sc_c,
            }
        )
    return in_maps


def kernel(inp, quant_weight, scales, zeros):
    from concourse.bass_utils import run_bass_kernel_spmd

    nc = _get_program()
    in_maps = _host_prep(inp, quant_weight, scales, zeros)
    res = run_bass_kernel_spmd(nc, in_maps, core_ids=list(range(NCORES)))
    out = np.concatenate(
        [res.results[c]["out"] for c in range(NCORES)], axis=1
    )
    return np.ascontiguousarray(out.astype(np.float32))
